# revision 1
# baseline (speedup 1.0000x reference)
"""Trainium2 Bass kernel for CenteredBilinearPooling (mean + strided windowed
covariance pooling).

Computation (matching the reference):
  x: [B=4, T=2048, C=128], w1, w2: [L=15]
  mu[t]     = sum_l w1[l] * xpad[t + l]          (xpad = zero-pad 7 both sides)
  xc        = x - mu
  sigma[t'] = sum_l w2[l] * outer(xcp[2t'+l], xcp[2t'+l])   (xcp = zero-pad 7)
  out       = concat(mu[:, ::2], sigma.reshape(B, T', C*C))  -> [4, 1024, 16512]

Distribution: 8 cores = (batch b in 0..3) x (time half h in 0..1); each core
computes 512 output frames. Full inputs are sliced/padded on host, outputs
gathered on host.

Per-core design (per 128-row time tile, advancing 88 rows = 44 windows):
  1. mu at the 44 strided output frames via a strided-band matmul
       m2 = A2^T @ X  (fp32, exact)
  2. window gathering, centering, edge masking and sqrt(w2) scaling are all
     folded into 13 host-precomputed constant "gather matrices" G_q:
       G_q = (I - A) @ diag(v) @ S_q
     where A is the mu band, v the edge-validity mask, and S_q selects the
     rows of windows 4q..4q+3 into 32-aligned partition slots scaled by
     sqrt(w2).  One matmul  stacked = G_q^T @ X  then yields 4 centered
     windows at partition bases {0,32,64,96} (matmul operands must start at
     partition 0/32/64/96 on TRN2, hence the slot layout).
  3. 4 window matmuls K=15 (lhsT = rhs = slot): sigma blocks land in one
     PSUM bank [128, 512]; DVE drains to an SBUF staging tile.
  4. One big DMA per tile writes the sigma columns of 44 output rows; mu
     rows are DMA'd from the m2 drain.
"""

import sys

if "/opt/trn_rl_repo" not in sys.path:
    sys.path.insert(0, "/opt/trn_rl_repo")

import numpy as np

B, T, C, L = 4, 2048, 128, 15
STRIDE = 2
TP = T // STRIDE            # 1024 output frames total
TPC = TP // 2               # 512 output frames per core
PAD = L // 2                # 7
ADV = 88                    # tile advance in time rows (44 windows * stride 2)
WPT = 44                    # windows per full tile
NTILES = 12                 # 11 full tiles + 1 tail tile (28 windows)
XIN_ROWS = ADV * (NTILES - 1) + 128   # 1096
OUT_COLS = C + C * C        # 16512
NG_FULL = WPT // 4          # 11 window groups per full tile
NG_TAIL = (TPC - WPT * (NTILES - 1)) // 4   # 7 groups in the tail tile
NGM = NG_FULL + 2           # 11 shared G matrices + 2 edge-masked variants

# window-path dtype: "fp16" (fast, ~1e-3 rel err), "bf16", or "fp32" (exact,
# 4x slower PE streams)
WINDOW_DTYPE = "fp16"
TRACE = False
LAST_RESULTS = {}

_cache = {}


def _g_index(k, q):
    """Which of the NGM gather matrices serves tile k, group q."""
    if k == 0 and q == 0:
        return NG_FULL          # edge-masked variant of G_0 (start of sequence)
    if k == NTILES - 1 and q == NG_TAIL - 1:
        return NG_FULL + 1      # edge-masked variant of G_6 (end of sequence)
    return q


def _build_program(wdt_name):
    return _build_program_repeat(wdt_name, 1)


def _build_program_repeat(wdt_name, repeats):
    import concourse.bacc as bacc
    import concourse.mybir as mybir
    import concourse.tile as tile

    f32 = mybir.dt.float32
    wdt = {"fp16": mybir.dt.float16, "bf16": mybir.dt.bfloat16, "fp32": f32}[wdt_name]

    nc = bacc.Bacc("TRN2", target_bir_lowering=False, debug=False)

    xin_t = nc.dram_tensor("xin", [XIN_ROWS, C], f32, kind="ExternalInput")
    a2_t = nc.dram_tensor("aband2", [128, WPT], f32, kind="ExternalInput")
    g_t = nc.dram_tensor("gmats", [NGM * 128, 128], wdt, kind="ExternalInput")
    out_t = nc.dram_tensor("out", [TPC, OUT_COLS], f32, kind="ExternalOutput")

    with tile.TileContext(nc) as tc:
        with (
            tc.tile_pool(name="const", bufs=1) as cpool,
            tc.tile_pool(name="xpool", bufs=3) as xpool,
            tc.tile_pool(name="work", bufs=2) as wpool,
            tc.tile_pool(name="spool", bufs=3) as spool,
            tc.tile_pool(name="stage", bufs=2) as stpool,
            tc.tile_pool(name="psA", bufs=2, space="PSUM") as psA,
            tc.tile_pool(name="psG", bufs=2, space="PSUM") as psG,
            tc.tile_pool(name="psP", bufs=2, space="PSUM") as psP,
        ):
            A2t = cpool.tile([128, WPT], f32)
            nc.sync.dma_start(A2t[:], a2_t.ap()[:, :])
            Gt = []
            for i in range(NGM):
                g = cpool.tile([128, 128], wdt, tag=f"g{i}")
                nc.sync.dma_start(g[:], g_t.ap()[128 * i : 128 * (i + 1), :])
                Gt.append(g)

            import contextlib

            loop_ctx = (
                tc.For_i(0, repeats, 1) if repeats > 1 else contextlib.nullcontext()
            )
            with loop_ctx:
              for k in range(NTILES):
                nwin = min(WPT, TPC - WPT * k)      # 44, tail tile 28
                ngrp = nwin // 4                    # 11, tail tile 7
                r0 = ADV * k

                X = xpool.tile([128, C], f32, tag="X")
                nc.sync.dma_start(X[:], xin_t.ap()[r0 : r0 + 128, :])

                m2_ps = psA.tile([WPT, 128], f32, tag="m2")
                nc.tensor.matmul(m2_ps[:], A2t[:], X[:], start=True, stop=True)
                mu_sb = wpool.tile([WPT, 128], f32, tag="mu")
                nc.vector.tensor_copy(mu_sb[:], m2_ps[:])
                nc.scalar.dma_start(
                    out_t.ap()[WPT * k : WPT * k + nwin, 0:C], mu_sb[:nwin, :]
                )

                if wdt_name == "fp32":
                    Xw = X
                else:
                    Xw = wpool.tile([128, C], wdt, tag="Xw")
                    nc.vector.tensor_copy(Xw[:], X[:])

                stage = stpool.tile([128, WPT * C], f32, tag="stage")
                for q in range(ngrp):
                    Gq = Gt[_g_index(k, q)]
                    g_ps = psG.tile([128, 128], f32, tag="gps")
                    nc.tensor.matmul(g_ps[:], Gq[:], Xw[:], start=True, stop=True)
                    S = spool.tile([128, 128], wdt, tag="S")
                    # S drains on ACT, sigma drains on DVE: moving sigma
                    # drains to ACT measured slower (ACT's sequencer also
                    # issues half the output DMAs)
                    nc.scalar.copy(S[:], g_ps[:])
                    for half in range(2):
                        # each matmul must own its PSUM bank (concurrent
                        # row-group matmuls draining into one bank crash
                        # TRN2); a [128, 1024] tile spans 2 banks, windows
                        # land in cols 0:128 and 512:640, drained in one op
                        P2 = psP.tile([128, 1024], f32, tag="P2")
                        for i in range(2):
                            g = 2 * half + i
                            nc.tensor.matmul(
                                P2[:, 512 * i : 512 * i + 128],
                                S[32 * g : 32 * g + L, :],
                                S[32 * g : 32 * g + L, :],
                                start=True,
                                stop=True,
                                tile_position=(32 * g, 0),
                            )
                        from concourse.bass import AP as _AP

                        p2ap = P2[:]
                        src = _AP(
                            p2ap.tensor, p2ap.offset, [[1024, 128], [512, 2], [1, 128]]
                        )
                        dst = stage[
                            :, 512 * q + 256 * half : 512 * q + 256 * (half + 1)
                        ].rearrange("c (i d) -> c i d", i=2)
                        nc.vector.tensor_copy(dst, src)

                # split the sigma store across both HWDGE rings
                half_w = (nwin + 1) // 2
                for part, eng in ((0, nc.sync), (1, nc.scalar)):
                    j0 = part * half_w
                    j1 = min(nwin, (part + 1) * half_w)
                    if j0 >= j1:
                        continue
                    eng.dma_start(
                        out_t.ap()[
                            WPT * k + j0 : WPT * k + j1, C:
                        ].rearrange("j (c d) -> c j d", c=C),
                        stage[:, j0 * C : j1 * C].rearrange(
                            "c (j d) -> c j d", d=C
                        ),
                    )

    nc.compile()
    return nc


def _host_constants(w1, w2, wdt_np):
    """Constants shared by all cores: A2 band and the unmasked pieces of G."""
    # mu band for centering: m[p] = sum_l w1[l] X[p - 7 + l]
    A = np.zeros((128, 128), np.float64)
    for p in range(128):
        for l in range(L):
            s = p - PAD + l
            if 0 <= s < 128:
                A[s, p] = w1[l]
    IA = np.eye(128) - A
    # strided band for mu at the 44 output frames: m2[p] = sum_l w1[l] X[2p+8+l]
    A2 = np.zeros((128, WPT), np.float32)
    for p in range(WPT):
        for l in range(L):
            s = 2 * p + 8 + l
            if 0 <= s < 128:
                A2[s, p] = w1[l]
    # window-selection matrices, sqrt(w2)-scaled:
    #   S_q[p, 32g + j] = sqrt(w2[j])  iff  j < 15 and p == 8 + 8q + 2g + j
    sq = np.sqrt(np.maximum(np.asarray(w2, np.float64), 0.0))
    Sq = []
    for q in range(NG_FULL):
        s0 = 8 + 8 * q
        Sm = np.zeros((128, 128), np.float64)
        for g in range(4):
            for j in range(L):
                Sm[s0 + 2 * g + j, 32 * g + j] = sq[j]
        Sq.append(Sm)
    return A2, IA, Sq


def _host_inputs(x, w1, w2, wdt_np):
    x = np.asarray(x, dtype=np.float32)
    w1 = np.asarray(w1, dtype=np.float64)
    w2 = np.asarray(w2, dtype=np.float64)
    A2, IA, Sq = _host_constants(w1, w2, wdt_np)

    # ext row e corresponds to padded-sequence (xpad) row e - 8;
    # xpad row r = x[r - 7] for 7 <= r < 7 + T else 0.
    in_maps = []
    for b in range(B):
        ext = np.zeros((2120, C), np.float32)
        ext[15 : 15 + T] = x[b]
        vext = np.zeros(2120, np.float64)
        vext[15 : 15 + T] = 1.0
        for h in range(2):
            base = 1024 * h
            xin = np.ascontiguousarray(ext[base : base + XIN_ROWS])
            gm = np.empty((NGM * 128, 128), wdt_np)
            for q in range(NG_FULL):
                gm[128 * q : 128 * (q + 1)] = (IA @ Sq[q]).astype(wdt_np)
            # edge-masked variants: diag(v) applied to the selected rows
            v0 = vext[base : base + 128]
            gm[128 * NG_FULL : 128 * (NG_FULL + 1)] = (
                IA @ (v0[:, None] * Sq[0])
            ).astype(wdt_np)
            r0_tail = ADV * (NTILES - 1)
            v11 = vext[base + r0_tail : base + r0_tail + 128]
            gm[128 * (NG_FULL + 1) :] = (
                IA @ (v11[:, None] * Sq[NG_TAIL - 1])
            ).astype(wdt_np)
            in_maps.append(
                {
                    "xin": xin,
                    "aband2": A2,
                    "gmats": np.ascontiguousarray(gm),
                }
            )
    return in_maps


def kernel(x, w1, w2):
    from concourse import bass_utils

    global LAST_RESULTS

    key = WINDOW_DTYPE
    if key not in _cache:
        _cache[key] = _build_program(key)
    nc = _cache[key]

    import ml_dtypes

    wdt_np = {
        "fp16": np.float16,
        "bf16": ml_dtypes.bfloat16,
        "fp32": np.float32,
    }[WINDOW_DTYPE]

    in_maps = _host_inputs(x, w1, w2, wdt_np)
    res = bass_utils.run_bass_kernel_spmd(
        nc, in_maps, core_ids=list(range(8)), trace=TRACE
    )
    LAST_RESULTS = {"exec_time_ns": res.exec_time_ns}

    out = np.empty((B, TP, OUT_COLS), np.float32)
    for core, r in enumerate(res.results):
        b, h = core // 2, core % 2
        out[b, 512 * h : 512 * (h + 1)] = r["out"]
    return out



# revision 3
# speedup vs baseline: 1.8004x; 1.8004x over previous
"""Trainium2 Bass kernel for CenteredBilinearPooling (mean + strided windowed
covariance pooling).

Computation (matching the reference):
  x: [B=4, T=2048, C=128], w1, w2: [L=15]
  mu[t]     = sum_l w1[l] * xpad[t + l]          (xpad = zero-pad 7 both sides)
  xc        = x - mu
  sigma[t'] = sum_l w2[l] * outer(xcp[2t'+l], xcp[2t'+l])   (xcp = zero-pad 7)
  out       = concat(mu[:, ::2], sigma.reshape(B, T', C*C))  -> [4, 1024, 16512]

Distribution: 8 cores = (batch b in 0..3) x (time half h in 0..1); each core
computes 512 output frames. Full inputs are sliced/padded on host, outputs
gathered on host.

Per-core design (per 128-row time tile, advancing 88 rows = 44 windows):
  1. mu at the 44 strided output frames via a strided-band matmul
       m2 = A2^T @ X  (fp32, exact)
  2. window gathering, centering, edge masking and sqrt(w2) scaling are all
     folded into 13 host-precomputed constant "gather matrices" G_q:
       G_q = (I - A) @ diag(v) @ S_q
     where A is the mu band, v the edge-validity mask, and S_q selects the
     rows of windows 4q..4q+3 into 32-aligned partition slots scaled by
     sqrt(w2).  One matmul  stacked = G_q^T @ X  then yields 4 centered
     windows at partition bases {0,32,64,96} (matmul operands must start at
     partition 0/32/64/96 on TRN2, hence the slot layout).
  3. 4 window matmuls K=15 (lhsT = rhs = slot): sigma blocks land in one
     PSUM bank [128, 512]; DVE drains to an SBUF staging tile.
  4. One big DMA per tile writes the sigma columns of 44 output rows; mu
     rows are DMA'd from the m2 drain.
"""

import sys

if "/opt/trn_rl_repo" not in sys.path:
    sys.path.insert(0, "/opt/trn_rl_repo")

import numpy as np

B, T, C, L = 4, 2048, 128, 15
STRIDE = 2
TP = T // STRIDE            # 1024 output frames total
TPC = TP // 2               # 512 output frames per core
PAD = L // 2                # 7
ADV = 88                    # tile advance in time rows (44 windows * stride 2)
WPT = 44                    # windows per full tile
NTILES = 12                 # 11 full tiles + 1 tail tile (28 windows)
XIN_ROWS = ADV * (NTILES - 1) + 128   # 1096
OUT_COLS = C + C * C        # 16512
NG_FULL = WPT // 4          # 11 window groups per full tile
NG_TAIL = (TPC - WPT * (NTILES - 1)) // 4   # 7 groups in the tail tile
NGM = NG_FULL + 2           # 11 shared G matrices + 2 edge-masked variants

# window-path dtype: "fp16" (fast, ~1e-3 rel err), "bf16", or "fp32" (exact,
# 4x slower PE streams)
WINDOW_DTYPE = "fp16"
TRACE = False
LAST_RESULTS = {}

_cache = {}


def _g_index(k, q):
    """Which of the NGM gather matrices serves tile k, group q."""
    if k == 0 and q == 0:
        return NG_FULL          # edge-masked variant of G_0 (start of sequence)
    if k == NTILES - 1 and q == NG_TAIL - 1:
        return NG_FULL + 1      # edge-masked variant of G_6 (end of sequence)
    return q


def _build_program(wdt_name):
    return _build_program_repeat(wdt_name, 1)


def _build_program_repeat(wdt_name, repeats):
    import concourse.bacc as bacc
    import concourse.mybir as mybir
    import concourse.tile as tile

    f32 = mybir.dt.float32
    wdt = {"fp16": mybir.dt.float16, "bf16": mybir.dt.bfloat16, "fp32": f32}[wdt_name]

    nc = bacc.Bacc("TRN2", target_bir_lowering=False, debug=False)

    xin_t = nc.dram_tensor("xin", [XIN_ROWS, C], f32, kind="ExternalInput")
    a2_t = nc.dram_tensor("aband2", [128, WPT], f32, kind="ExternalInput")
    g_t = nc.dram_tensor("gmats", [NGM * 128, 128], wdt, kind="ExternalInput")
    out_t = nc.dram_tensor("out", [TPC, OUT_COLS], f32, kind="ExternalOutput")

    with tile.TileContext(nc) as tc:
        with (
            tc.tile_pool(name="const", bufs=1) as cpool,
            tc.tile_pool(name="xpool", bufs=3) as xpool,
            tc.tile_pool(name="work", bufs=2) as wpool,
            tc.tile_pool(name="spool", bufs=3) as spool,
            tc.tile_pool(name="stage", bufs=2) as stpool,
            tc.tile_pool(name="psA", bufs=2, space="PSUM") as psA,
            tc.tile_pool(name="psG", bufs=2, space="PSUM") as psG,
            tc.tile_pool(name="psP", bufs=2, space="PSUM") as psP,
        ):
            A2t = cpool.tile([128, WPT], f32)
            nc.sync.dma_start(A2t[:], a2_t.ap()[:, :])
            Gt = []
            for i in range(NGM):
                g = cpool.tile([128, 128], wdt, tag=f"g{i}")
                nc.sync.dma_start(g[:], g_t.ap()[128 * i : 128 * (i + 1), :])
                Gt.append(g)

            import contextlib

            loop_ctx = (
                tc.For_i(0, repeats, 1) if repeats > 1 else contextlib.nullcontext()
            )
            with loop_ctx:
              for k in range(NTILES):
                nwin = min(WPT, TPC - WPT * k)      # 44, tail tile 28
                ngrp = nwin // 4                    # 11, tail tile 7
                r0 = ADV * k

                X = xpool.tile([128, C], f32, tag="X")
                nc.sync.dma_start(X[:], xin_t.ap()[r0 : r0 + 128, :])

                m2_ps = psA.tile([WPT, 128], f32, tag="m2")
                nc.tensor.matmul(m2_ps[:], A2t[:], X[:], start=True, stop=True)
                mu_sb = wpool.tile([WPT, 128], f32, tag="mu")
                nc.vector.tensor_copy(mu_sb[:], m2_ps[:])
                # mu rides the queue the sigma store of this tile does NOT use
                [nc.scalar, nc.sync][k % 2].dma_start(
                    out_t.ap()[WPT * k : WPT * k + nwin, 0:C], mu_sb[:nwin, :]
                )

                if wdt_name == "fp32":
                    Xw = X
                else:
                    Xw = wpool.tile([128, C], wdt, tag="Xw")
                    nc.vector.tensor_copy(Xw[:], X[:])

                stage = stpool.tile([128, WPT * C], f32, tag="stage")
                for q in range(ngrp):
                    Gq = Gt[_g_index(k, q)]
                    g_ps = psG.tile([128, 128], f32, tag="gps")
                    nc.tensor.matmul(g_ps[:], Gq[:], Xw[:], start=True, stop=True)
                    S = spool.tile([128, 128], wdt, tag="S")
                    # S drains on ACT, sigma drains on DVE: moving sigma
                    # drains to ACT measured slower (ACT's sequencer also
                    # issues half the output DMAs)
                    nc.scalar.copy(S[:], g_ps[:])
                    for half in range(2):
                        # each matmul must own its PSUM bank (concurrent
                        # row-group matmuls draining into one bank crash
                        # TRN2); a [128, 1024] tile spans 2 banks, windows
                        # land in cols 0:128 and 512:640, drained in one op
                        P2 = psP.tile([128, 1024], f32, tag="P2")
                        for i in range(2):
                            g = 2 * half + i
                            nc.tensor.matmul(
                                P2[:, 512 * i : 512 * i + 128],
                                S[32 * g : 32 * g + L, :],
                                S[32 * g : 32 * g + L, :],
                                start=True,
                                stop=True,
                                tile_position=(32 * g, 0),
                            )
                        from concourse.bass import AP as _AP

                        p2ap = P2[:]
                        src = _AP(
                            p2ap.tensor, p2ap.offset, [[1024, 128], [512, 2], [1, 128]]
                        )
                        dst = stage[
                            :, 512 * q + 256 * half : 512 * q + 256 * (half + 1)
                        ].rearrange("c (i d) -> c i d", i=2)
                        nc.vector.tensor_copy(dst, src)

                # one whole-tile sigma store, alternating the two HWDGE
                # rings per tile: A/B-measured ~1.5x faster than splitting
                # each tile across both rings (two concurrent DMAs over the
                # same 128 source partitions contend; alternation keeps one
                # clean stream per ring while the other ring fills)
                [nc.sync, nc.scalar][k % 2].dma_start(
                    out_t.ap()[WPT * k : WPT * k + nwin, C:].rearrange(
                        "j (c d) -> c j d", c=C
                    ),
                    stage[:, : nwin * C].rearrange("c (j d) -> c j d", d=C),
                )

    nc.compile()
    return nc


def _host_constants(w1, w2, wdt_np):
    """Constants shared by all cores: A2 band and the unmasked pieces of G."""
    # mu band for centering: m[p] = sum_l w1[l] X[p - 7 + l]
    A = np.zeros((128, 128), np.float64)
    for p in range(128):
        for l in range(L):
            s = p - PAD + l
            if 0 <= s < 128:
                A[s, p] = w1[l]
    IA = np.eye(128) - A
    # strided band for mu at the 44 output frames: m2[p] = sum_l w1[l] X[2p+8+l]
    A2 = np.zeros((128, WPT), np.float32)
    for p in range(WPT):
        for l in range(L):
            s = 2 * p + 8 + l
            if 0 <= s < 128:
                A2[s, p] = w1[l]
    # window-selection matrices, sqrt(w2)-scaled:
    #   S_q[p, 32g + j] = sqrt(w2[j])  iff  j < 15 and p == 8 + 8q + 2g + j
    sq = np.sqrt(np.maximum(np.asarray(w2, np.float64), 0.0))
    Sq = []
    for q in range(NG_FULL):
        s0 = 8 + 8 * q
        Sm = np.zeros((128, 128), np.float64)
        for g in range(4):
            for j in range(L):
                Sm[s0 + 2 * g + j, 32 * g + j] = sq[j]
        Sq.append(Sm)
    return A2, IA, Sq


def _host_inputs(x, w1, w2, wdt_np):
    x = np.asarray(x, dtype=np.float32)
    w1 = np.asarray(w1, dtype=np.float64)
    w2 = np.asarray(w2, dtype=np.float64)
    A2, IA, Sq = _host_constants(w1, w2, wdt_np)

    # ext row e corresponds to padded-sequence (xpad) row e - 8;
    # xpad row r = x[r - 7] for 7 <= r < 7 + T else 0.
    in_maps = []
    for b in range(B):
        ext = np.zeros((2120, C), np.float32)
        ext[15 : 15 + T] = x[b]
        vext = np.zeros(2120, np.float64)
        vext[15 : 15 + T] = 1.0
        for h in range(2):
            base = 1024 * h
            xin = np.ascontiguousarray(ext[base : base + XIN_ROWS])
            gm = np.empty((NGM * 128, 128), wdt_np)
            for q in range(NG_FULL):
                gm[128 * q : 128 * (q + 1)] = (IA @ Sq[q]).astype(wdt_np)
            # edge-masked variants: diag(v) applied to the selected rows
            v0 = vext[base : base + 128]
            gm[128 * NG_FULL : 128 * (NG_FULL + 1)] = (
                IA @ (v0[:, None] * Sq[0])
            ).astype(wdt_np)
            r0_tail = ADV * (NTILES - 1)
            v11 = vext[base + r0_tail : base + r0_tail + 128]
            gm[128 * (NG_FULL + 1) :] = (
                IA @ (v11[:, None] * Sq[NG_TAIL - 1])
            ).astype(wdt_np)
            in_maps.append(
                {
                    "xin": xin,
                    "aband2": A2,
                    "gmats": np.ascontiguousarray(gm),
                }
            )
    return in_maps


def kernel(x, w1, w2):
    from concourse import bass_utils

    global LAST_RESULTS

    key = WINDOW_DTYPE
    if key not in _cache:
        _cache[key] = _build_program(key)
    nc = _cache[key]

    import ml_dtypes

    wdt_np = {
        "fp16": np.float16,
        "bf16": ml_dtypes.bfloat16,
        "fp32": np.float32,
    }[WINDOW_DTYPE]

    in_maps = _host_inputs(x, w1, w2, wdt_np)
    res = bass_utils.run_bass_kernel_spmd(
        nc, in_maps, core_ids=list(range(8)), trace=TRACE
    )
    LAST_RESULTS = {"exec_time_ns": res.exec_time_ns}

    out = np.empty((B, TP, OUT_COLS), np.float32)
    for core, r in enumerate(res.results):
        b, h = core // 2, core % 2
        out[b, 512 * h : 512 * (h + 1)] = r["out"]
    return out

